# revision 48
# baseline (speedup 1.0000x reference)
"""Multi-head self-attention (RoPE + softmax + out-proj) for Trainium2,
sharded over 8 NeuronCores: data-parallel over batch (4) x tensor-parallel
over heads (2 groups of 8). Each core computes q/k/v projections for its
head group, attention, and a partial output projection; the host sums the
two partials per batch and adds the bias.

fp16 matmul operands (host-cast; 1 cycle/row on the PE without the
fp32-HIGH power mode), x^T and v resident in SBUF (no DRAM bounces), and a
schedule that keeps the PE dense so the HAM clock gate stays open:
  - the v/k/q projection pass is merged with pair-0 quarter-0 attention,
    with v-stage copies on the (otherwise idle) scalar engine and q blocks
    emitted a quarter ahead of use;
  - the softmax normalize tail (reciprocal + broadcast + scale) is
    deferred to the middle of the NEXT quarter -- nothing reads otn[q]
    until the pair-3 out-projection, so the slow reciprocal never sits on
    the PE's critical path;
  - pair p+1 projections / finished out-projections drain as fillers at a
    fixed per-quarter rate.
"""

import numpy as np

import concourse.bass as bass
import concourse.mybir as mybir
import concourse.tile as tile

B, N, DIM, H, DH = 4, 2048, 1024, 16, 64
SCALE = DH**-0.5
N_CORES = 8
HG = 8  # heads per core
INNER = HG * DH  # 512, inner dim slice per core
PAIRS = INNER // 128  # 4 head pairs (=128-partition inner chunks)
NB = 4  # n blocks of 512
MB = 16  # m blocks of 128
KD = DIM // 128  # 8 contraction chunks

F32 = mybir.dt.float32
F32R = mybir.dt.float32r
F16 = mybir.dt.float16
EXP = mybir.ActivationFunctionType.Exp

MAX_WAITS = 1


def _split_excess_waits(nc):
    """This walrus build rejects >1 semaphore wait per instruction; hoist
    excess waits onto nops inserted before the instruction on its engine."""
    import bass_rust

    for f in nc.m.functions:
        for bb in f.blocks:
            il = bb.instructions
            i = 0
            while i < len(il):
                inst = il[i]
                si = inst.sync_info
                if si is not None and si.on_wait and len(si.on_wait) > MAX_WAITS:
                    waits = list(si.on_wait)
                    si.on_wait = waits[:MAX_WAITS]
                    rest = waits[MAX_WAITS:]
                    eng = nc.engines[inst.engine]
                    insert_at = i
                    for j in range(0, len(rest), MAX_WAITS):
                        b = eng.nop(nofuse=True, hint="wait_split")
                        ni = b.ins
                        tail = nc.cur_bb.bb.instructions
                        assert tail[-1] is ni
                        tail.pop()
                        nsi = ni.sync_info
                        if nsi is None:
                            ni.sync_info = bass_rust.SyncInfo(
                                on_wait=rest[j : j + MAX_WAITS], on_update=[]
                            )
                        else:
                            nsi.on_wait = rest[j : j + MAX_WAITS]
                        il.insert(insert_at, ni)
                        insert_at += 1
                        i += 1
                i += 1


class _FixedTileContext(tile.TileContext):
    def __exit__(self, exc_type, exc_val, exc_tb):
        res = super().__exit__(exc_type, exc_val, exc_tb)
        if exc_type is None:
            _split_excess_waits(self.nc)
        return res


def build_kernel():
    nc = bass.Bass()
    xT = nc.dram_tensor("xT", [DIM, N], F16, kind="ExternalInput")
    wq = nc.dram_tensor("wq", [DIM, INNER], F16, kind="ExternalInput")
    wk = nc.dram_tensor("wk", [DIM, INNER], F16, kind="ExternalInput")
    wv = nc.dram_tensor("wv", [DIM, INNER], F16, kind="ExternalInput")
    wo = nc.dram_tensor("wo", [INNER, DIM], F16, kind="ExternalInput")
    cosT = nc.dram_tensor("cosT", [128, N], F16, kind="ExternalInput")
    sinT = nc.dram_tensor("sinT", [128, N], F16, kind="ExternalInput")
    out = nc.dram_tensor("out", [N, DIM], F32, kind="ExternalOutput")

    xTr = xT.rearrange("(c p) n -> p c n", p=128)
    wvr = wv.rearrange("(c p) i -> p c i", p=128)

    with _FixedTileContext(nc) as tc:
        with (
            tc.tile_pool(name="const", bufs=1) as cpool,
            tc.tile_pool(name="qk", bufs=1) as qkpool,
            tc.tile_pool(name="ps", space=bass.MemorySpace.PSUM, bufs=1) as ps,
            tc.tile_pool(name="io", bufs=1) as iopool,
        ):
            # ---- resident tensors / constants ----
            xb = cpool.tile([128, KD, N], F16, tag="xb")
            wv_t = cpool.tile([128, KD, INNER], F16, tag="wv")
            # split the first-needed DMAs in dc chunks across two queues so
            # the first v matmul can start as soon as chunk 0 lands; all of
            # x is prefetched up front, balanced across the queues
            for dc in range(KD):
                nc.sync.dma_start(wv_t[:, dc, :], wvr[:, dc, :])
                nc.gpsimd.dma_start(xb[:, dc, 0:512], xTr[:, dc, 0:512])
            cos_t = cpool.tile([128, N], F16, tag="cos")
            sin_t = cpool.tile([128, N], F16, tag="sin")

            ve = [
                cpool.tile([128, PAIRS, MB, 65], F16, tag=f"ve{j}", name=f"ve{j}")
                for j in range(2)
            ]
            for j in range(2):
                nc.gpsimd.memset(ve[j][:, :, :, 64:65], 1.0)
            # ones rows for the denominator-broadcast matmul (K=1, M=64);
            # rows 0 and 32 are used (matching rec's row layout). The value
            # is 1/64 because rec is prescaled by 64 to keep 64/D in fp16's
            # normal range (1/D can go subnormal); bc = (1/64)*(64/D) = 1/D.
            ones16 = cpool.tile([33, 64], F16, tag="ones16")
            nc.vector.memset(ones16[:], 1.0 / 64.0)

            # ---- per-pair q/k projection blocks ----
            def proj_pair_blocks(p):
                csl = slice(p * 128, (p + 1) * 128)
                wt = {}

                def load_w():
                    for nm, wd in (("q", wq), ("k", wk)):
                        t = iopool.tile(
                            [128, KD, 128], F16, tag=f"w{nm}", bufs=2, name=f"w{nm}_{p}"
                        )
                        nc.sync.dma_start(
                            t[:], wd.rearrange("(c p) i -> p c i", p=128)[:, :, csl]
                        )
                        wt[nm] = t

                qT_t = qkpool.tile([128, N], F16, tag="qT", bufs=2)
                kT_t = qkpool.tile([128, N], F16, tag="kT", bufs=2)

                def block(nb, nm, tgt):
                    def emit():
                        nsl = slice(nb * 512, (nb + 1) * 512)
                        pq = ps.tile([128, 2, 512], F32, tag="s", bufs=3)
                        for dc in range(KD):
                            nc.tensor.matmul(
                                pq[:, 0, :], wt[nm][:, dc, :], xb[:, dc, nsl],
                                start=(dc == 0), stop=(dc == KD - 1),
                            )
                        # rotate_half via 32-partition shifted copies; sign
                        # folded into sin_t (host negates alternate groups)
                        tmp = iopool.tile([128, 512], F16, tag="tmp", bufs=2)
                        for g in range(4):
                            dst = slice(g * 32, (g + 1) * 32)
                            ssrc = slice((g ^ 1) * 32, ((g ^ 1) + 1) * 32)
                            nc.vector.tensor_copy(tmp[dst, :], pq[ssrc, 0, :])
                        nc.vector.tensor_mul(tmp[:], tmp[:], sin_t[:, nsl])
                        nc.vector.tensor_mul(tgt[:, nsl], pq[:, 0, :], cos_t[:, nsl])
                        nc.vector.tensor_add(tgt[:, nsl], tgt[:, nsl], tmp[:])

                    return emit

                blocks = {}
                for nb in range(NB):
                    blocks[("q", nb)] = block(nb, "q", qT_t)
                    blocks[("k", nb)] = block(nb, "k", kT_t)
                return load_w, blocks, qT_t, kT_t

            with tc.tile_pool(name="attn", bufs=1) as at:
                otn = [
                    at.tile([128, 4, 512], F16, tag=f"otn{p}", name=f"otn{p}")
                    for p in range(PAIRS)
                ]
                wo_h = []

                def load_wo():
                    for dh in range(2):
                        woh = qkpool.tile(
                            [128, PAIRS, 512], F16, tag=("qT", "kT")[dh], bufs=2,
                            name=f"wo_h{dh}",
                        )
                        nc.sync.dma_start(
                            woh[:],
                            wo.rearrange("(c p) d -> p c d", p=128)[
                                :, :, dh * 512 : (dh + 1) * 512
                            ],
                        )
                        wo_h.append(woh)

                fillers = []

                def drain(k=1):
                    for _ in range(k):
                        if fillers:
                            fillers.pop(0)()

                pending = []  # deferred normalize tails

                def run_pending():
                    while pending:
                        pending.pop(0)()

                def outproj_block(nbk, dh):
                    def emit():
                        q4, r4 = divmod(nbk, 4)
                        nsl = slice(nbk * 128, (nbk + 1) * 128)
                        po = ps.tile([128, 2, 512], F32, tag="s", bufs=3)
                        for c in range(PAIRS):
                            nc.tensor.matmul(
                                po[:, 0, :],
                                otn[c][:, q4, r4 * 128 : (r4 + 1) * 128],
                                wo_h[dh][:, c, :],
                                start=(c == 0),
                                stop=(c == PAIRS - 1),
                            )
                        ost = iopool.tile([128, 512], F32, tag="ost", bufs=2)
                        nc.vector.tensor_copy(ost[:], po[:, 0, :])
                        nc.sync.dma_start(
                            out[nsl, dh * 512 : (dh + 1) * 512], ost[:]
                        )

                    return emit

                # ---- attention step: exp j0 is issued before scores j1 so
                # the scalar engine gets a head start on the av matmuls ----
                def scores_exp_av(p, qi, mb2, qT_t, kT_t, ot_ab):
                    n0 = qi * 512
                    pts = []
                    for j in range(2):
                        psl = slice(64 * j, 64 * (j + 1))
                        s_t = ps.tile(
                            [128, 2, 512], F32, tag="s", bufs=3, name=f"s{j}"
                        )
                        for hm in range(2):
                            mb = 2 * mb2 + hm
                            msl = slice(mb * 128, (mb + 1) * 128)
                            nc.tensor.matmul(
                                s_t[:, hm, :],
                                kT_t[psl, msl],
                                qT_t[psl, n0 : n0 + 512],
                                start=True,
                                stop=True,
                            )
                        pt = at.tile(
                            [128, 2, 512], F16, tag="pt", bufs=8, name=f"pt{j}"
                        )
                        nc.scalar.activation(pt[:], s_t[:], EXP, scale=SCALE)
                        pts.append(pt)
                    for j in range(2):
                        for hm in range(2):
                            mb = 2 * mb2 + hm
                            nc.tensor.matmul(
                                ot_ab[j][0:65, :],
                                ve[j][:, p, mb, :],
                                pts[j][:, hm, :],
                                start=(mb == 0),
                                stop=(mb == MB - 1),
                            )

                def spill_quarter(p, qi, ot_ab, last=False):
                    """Copy accumulators off PSUM at quarter end; defer the
                    reciprocal/broadcast/scale tail off the critical path."""
                    osb = at.tile([65, 2, 512], F16, tag="ots", bufs=2)
                    nc.vector.tensor_copy(osb[:, 0, :], ot_ab[0][0:65, :])
                    nc.vector.tensor_copy(osb[:, 1, :], ot_ab[1][0:65, :])

                    def tail():
                        rin = at.tile([33, 512], F32, tag="rin", bufs=2)
                        nc.vector.tensor_copy(rin[0:1, :], osb[64:65, 0, :])
                        nc.vector.tensor_copy(rin[32:33, :], osb[64:65, 1, :])
                        rec = at.tile([33, 512], F32R, tag="rec", bufs=2)
                        with nc.allow_low_precision(
                            reason="f32r reciprocal for softmax denom"
                        ):
                            nc.vector.reciprocal(rec[:], rin[:])
                        rec16 = at.tile([33, 512], F16, tag="rec16", bufs=2)
                        nc.vector.tensor_scalar_mul(rec16[:], rec[:], 64.0)
                        bcs = ps.tile(
                            [128, 2, 512], F32, tag="s", bufs=3, name="bcs"
                        )
                        for j in range(2):
                            row = 32 * j
                            nc.tensor.matmul(
                                bcs[0:64, j, :],
                                ones16[row : row + 1, :],
                                rec16[row : row + 1, :],
                                start=True,
                                stop=True,
                            )
                        for j in range(2):
                            nc.vector.tensor_mul(
                                otn[p][64 * j : 64 * (j + 1), qi, :],
                                osb[0:64, j, :],
                                bcs[0:64, j, :],
                            )
                        if p == PAIRS - 1:
                            for r4 in range(4):
                                for dh in range(2):
                                    fillers.append(outproj_block(qi * 4 + r4, dh))

                    def tail_last():
                        # split in n-halves so the final out-projections
                        # pipeline with the reciprocal chain
                        rin = at.tile([33, 512], F32, tag="rin", bufs=2)
                        nc.vector.tensor_copy(rin[0:1, :], osb[64:65, 0, :])
                        nc.vector.tensor_copy(rin[32:33, :], osb[64:65, 1, :])
                        rec = at.tile([33, 512], F32R, tag="rec", bufs=2)
                        rec16 = at.tile([33, 512], F16, tag="rec16", bufs=2)
                        bcs = ps.tile(
                            [128, 2, 512], F32, tag="s", bufs=3, name="bcs"
                        )
                        for h in range(2):
                            hs = slice(h * 256, (h + 1) * 256)
                            with nc.allow_low_precision(
                                reason="f32r reciprocal for softmax denom"
                            ):
                                nc.vector.reciprocal(rec[:, hs], rin[:, hs])
                            nc.vector.tensor_scalar_mul(
                                rec16[:, hs], rec[:, hs], 64.0
                            )
                            for j in range(2):
                                row = 32 * j
                                nc.tensor.matmul(
                                    bcs[0:64, j, hs],
                                    ones16[row : row + 1, :],
                                    rec16[row : row + 1, hs],
                                    start=True,
                                    stop=True,
                                )
                                nc.vector.tensor_mul(
                                    otn[p][64 * j : 64 * (j + 1), qi, hs],
                                    osb[0:64, j, hs],
                                    bcs[0:64, j, hs],
                                )
                            for r4 in (2 * h, 2 * h + 1):
                                for dh in range(2):
                                    outproj_block(qi * 4 + r4, dh)()

                    if last:
                        tail_last()
                    else:
                        pending.append(tail)

                # ---- pair 0: merged v/k/q pass + quarter-0 attention ----
                load_w0, blocks0, qT0, kT0 = proj_pair_blocks(0)
                load_w0()
                # prefetch the rest of x + rope tables behind the weights,
                # spread across four DMA queues (vector/scalar are idle here)
                nc.sync.dma_start(cos_t[:], cosT[:])
                nc.gpsimd.dma_start(sin_t[:], sinT[:])
                nc.scalar.dma_start(xb[:, :, 512:1024], xTr[:, :, 512:1024])
                nc.gpsimd.dma_start(xb[:, :, 1024:1536], xTr[:, :, 1024:1536])
                nc.sync.dma_start(xb[:, :, 1536:2048], xTr[:, :, 1536:2048])
                ot_q0 = [
                    ps.tile([128, 512], F32, tag="ot", bufs=2, name=f"ot{jj}")
                    for jj in range(2)
                ]
                # q blocks are emitted a quarter ahead of their use; the
                # quarter-0 attention lags one nb behind the k/v it needs so
                # scores never wait on just-emitted RoPE work
                q_sched = {0: [0], 1: [], 2: [1], 3: [2]}
                attn_sched = {0: (), 1: (0, 1), 2: (2, 3), 3: (4, 5)}
                for nb in range(NB):
                    for s2 in range(2):
                        pv = ps.tile([128, 2, 512], F32, tag="s", bufs=3)
                        for half in range(2):
                            mb = nb * 4 + s2 * 2 + half
                            m0 = mb * 128
                            for dc in range(KD):
                                nc.tensor.matmul(
                                    pv[:, half, :],
                                    xb[:, dc, m0 : m0 + 128],
                                    wv_t[:, dc, :],
                                    start=(dc == 0),
                                    stop=(dc == KD - 1),
                                )
                        pvr = pv[:].rearrange(
                            "p h (c j d) -> p h c j d", c=4, j=2, d=64
                        )
                        for half in range(2):
                            mb = nb * 4 + s2 * 2 + half
                            for j in range(2):
                                # scalar engine is idle until exp ramps up
                                nc.scalar.copy(
                                    ve[j][:, :, mb, 0:64], pvr[:, half, :, j, :]
                                )
                    blocks0[("k", nb)]()
                    for qb in q_sched[nb]:
                        blocks0[("q", qb)]()
                    for mb2 in attn_sched[nb]:
                        scores_exp_av(0, 0, mb2, qT0, kT0, ot_q0)
                for mb2 in (6, 7):
                    scores_exp_av(0, 0, mb2, qT0, kT0, ot_q0)
                spill_quarter(0, 0, ot_q0)

                pair_qk = {0: (qT0, kT0, blocks0)}

                # ---- remaining quarters / pairs ----
                for p in range(PAIRS):
                    qT_t, kT_t, blocks_p = pair_qk.pop(p)
                    if p == PAIRS - 1:
                        load_wo()
                    if p + 1 < PAIRS:
                        load_wn, blocks_n, qTn, kTn = proj_pair_blocks(p + 1)
                        load_wn()
                        pair_qk[p + 1] = (qTn, kTn, blocks_n)
                        new_fill = [
                            blocks_n[(nm, nb)] for nb in range(NB) for nm in ("k", "q")
                        ]
                        if p == 0:
                            # pair 0's own last q block goes first
                            new_fill.insert(0, blocks_p[("q", 3)])
                        fillers.extend(new_fill)
                    quarters = range(1, 4) if p == 0 else range(4)
                    for qi in quarters:
                        if p == 0:
                            drain_at = (1, 4, 6)
                            dk = 1
                        elif p < PAIRS - 1:
                            drain_at = (2, 5)
                            dk = 1
                        else:
                            drain_at = (1, 2, 3, 5, 6, 7)
                            dk = 2
                        ot_ab = [
                            ps.tile(
                                [128, 512], F32, tag="ot", bufs=2, name=f"ot{jj}"
                            )
                            for jj in range(2)
                        ]
                        # keep the tail's DVE burst off the filler-drain steps
                        tail_mb2 = 2 if p == 0 else 4
                        for mb2 in range(MB // 2):
                            scores_exp_av(p, qi, mb2, qT_t, kT_t, ot_ab)
                            if mb2 == tail_mb2:
                                run_pending()
                            if mb2 in drain_at:
                                drain(dk)
                        spill_quarter(
                            p, qi, ot_ab, last=(p == PAIRS - 1 and qi == 3)
                        )
                        if p == PAIRS - 1 and qi == 3:
                            drain(len(fillers))

    return nc


_CACHED = {}


def _get_kernel():
    if "nc" not in _CACHED:
        _CACHED["nc"] = build_kernel()
    return _CACHED["nc"]


def kernel(x, rotary_emb_x, Wq, Wkv, Wo, bo):
    from concourse.bass_utils import run_bass_kernel_spmd

    x = np.asarray(x, np.float32)
    rope = np.asarray(rotary_emb_x, np.float32)
    Wq = np.asarray(Wq, np.float32)
    Wkv = np.asarray(Wkv, np.float32)
    Wo = np.asarray(Wo, np.float32)
    bo = np.asarray(bo, np.float32)

    cosT = np.ascontiguousarray(np.cos(rope).T)  # [64, N]
    sinT = np.ascontiguousarray(np.sin(rope).T)
    cosT2 = np.concatenate([cosT, cosT], axis=0)
    sinT2 = np.concatenate([sinT, sinT], axis=0)
    # fold rotate_half's sign into sin: the low half of each 64-row head
    # block multiplies -q_hi
    sinT2 = sinT2.copy()
    sinT2[0:32] = -sinT2[0:32]
    sinT2[64:96] = -sinT2[64:96]
    cosT2 = np.ascontiguousarray(cosT2.astype(np.float16))
    sinT2 = np.ascontiguousarray(sinT2.astype(np.float16))

    Wk_full = Wkv[:, : H * DH]
    Wv_full = Wkv[:, H * DH :]

    xTs = [np.ascontiguousarray(x[b].T.astype(np.float16)) for b in range(B)]
    in_maps = []
    for core in range(N_CORES):
        b, hg = divmod(core, 2)
        isl = slice(hg * INNER, (hg + 1) * INNER)
        in_maps.append(
            {
                "xT": xTs[b],
                "wq": np.ascontiguousarray(Wq[:, isl].astype(np.float16)),
                "wk": np.ascontiguousarray(Wk_full[:, isl].astype(np.float16)),
                "wv": np.ascontiguousarray(Wv_full[:, isl].astype(np.float16)),
                "wo": np.ascontiguousarray(Wo[isl, :].astype(np.float16)),
                "cosT": cosT2,
                "sinT": sinT2,
            }
        )

    nc = _get_kernel()
    _CACHED["in_maps"] = in_maps
    res = run_bass_kernel_spmd(nc, in_maps, list(range(N_CORES)))
    outs = [res.results[i]["out"] for i in range(N_CORES)]
    full = np.stack(
        [outs[2 * b] + outs[2 * b + 1] + bo for b in range(B)], axis=0
    )
    return full


# revision 50
# speedup vs baseline: 1.0328x; 1.0328x over previous
"""Multi-head self-attention (RoPE + softmax + out-proj) for Trainium2,
sharded over 8 NeuronCores: data-parallel over batch (4) x tensor-parallel
over heads (2 groups of 8). Each core computes q/k/v projections for its
head group, attention, and a partial output projection; the host sums the
two partials per batch and adds the bias.

fp16 matmul operands (host-cast; 1 cycle/row on the PE without the
fp32-HIGH power mode), x^T and v resident in SBUF (no DRAM bounces), and a
schedule that keeps the PE dense so the HAM clock gate stays open:
  - the v/k/q projection pass is merged with pair-0 quarter-0 attention,
    with v-stage copies on the (otherwise idle) scalar engine and q blocks
    emitted a quarter ahead of use;
  - the softmax normalize tail (reciprocal + broadcast + scale) is
    deferred to the middle of the NEXT quarter -- nothing reads otn[q]
    until the pair-3 out-projection, so the slow reciprocal never sits on
    the PE's critical path;
  - pair p+1 projections / finished out-projections drain as fillers at a
    fixed per-quarter rate.
"""

import numpy as np

import concourse.bass as bass
import concourse.mybir as mybir
import concourse.tile as tile

B, N, DIM, H, DH = 4, 2048, 1024, 16, 64
SCALE = DH**-0.5
N_CORES = 8
HG = 8  # heads per core
INNER = HG * DH  # 512, inner dim slice per core
PAIRS = INNER // 128  # 4 head pairs (=128-partition inner chunks)
NB = 4  # n blocks of 512
MB = 16  # m blocks of 128
KD = DIM // 128  # 8 contraction chunks

F32 = mybir.dt.float32
F32R = mybir.dt.float32r
F16 = mybir.dt.float16
EXP = mybir.ActivationFunctionType.Exp

MAX_WAITS = 1


def _split_excess_waits(nc):
    """This walrus build rejects >1 semaphore wait per instruction; hoist
    excess waits onto nops inserted before the instruction on its engine."""
    import bass_rust

    for f in nc.m.functions:
        for bb in f.blocks:
            il = bb.instructions
            i = 0
            while i < len(il):
                inst = il[i]
                si = inst.sync_info
                if si is not None and si.on_wait and len(si.on_wait) > MAX_WAITS:
                    waits = list(si.on_wait)
                    si.on_wait = waits[:MAX_WAITS]
                    rest = waits[MAX_WAITS:]
                    eng = nc.engines[inst.engine]
                    insert_at = i
                    for j in range(0, len(rest), MAX_WAITS):
                        b = eng.nop(nofuse=True, hint="wait_split")
                        ni = b.ins
                        tail = nc.cur_bb.bb.instructions
                        assert tail[-1] is ni
                        tail.pop()
                        nsi = ni.sync_info
                        if nsi is None:
                            ni.sync_info = bass_rust.SyncInfo(
                                on_wait=rest[j : j + MAX_WAITS], on_update=[]
                            )
                        else:
                            nsi.on_wait = rest[j : j + MAX_WAITS]
                        il.insert(insert_at, ni)
                        insert_at += 1
                        i += 1
                i += 1


class _FixedTileContext(tile.TileContext):
    def __exit__(self, exc_type, exc_val, exc_tb):
        res = super().__exit__(exc_type, exc_val, exc_tb)
        if exc_type is None:
            _split_excess_waits(self.nc)
        return res


def build_kernel():
    nc = bass.Bass()
    xT = nc.dram_tensor("xT", [DIM, N], F16, kind="ExternalInput")
    wq = nc.dram_tensor("wq", [DIM, INNER], F16, kind="ExternalInput")
    wk = nc.dram_tensor("wk", [DIM, INNER], F16, kind="ExternalInput")
    wv = nc.dram_tensor("wv", [DIM, INNER], F16, kind="ExternalInput")
    wo = nc.dram_tensor("wo", [INNER, DIM], F16, kind="ExternalInput")
    cosT = nc.dram_tensor("cosT", [128, N], F16, kind="ExternalInput")
    sinT = nc.dram_tensor("sinT", [128, N], F16, kind="ExternalInput")
    out = nc.dram_tensor("out", [N, DIM], F32, kind="ExternalOutput")

    xTr = xT.rearrange("(c p) n -> p c n", p=128)
    wvr = wv.rearrange("(c p) i -> p c i", p=128)

    with _FixedTileContext(nc) as tc:
        with (
            tc.tile_pool(name="const", bufs=1) as cpool,
            tc.tile_pool(name="qk", bufs=1) as qkpool,
            tc.tile_pool(name="ps", space=bass.MemorySpace.PSUM, bufs=1) as ps,
            tc.tile_pool(name="io", bufs=1) as iopool,
        ):
            # ---- resident tensors / constants ----
            xb = cpool.tile([128, KD, N], F16, tag="xb")
            wv_t = cpool.tile([128, KD, INNER], F16, tag="wv")
            # split the first-needed DMAs in dc chunks across two queues so
            # the first v matmul can start as soon as chunk 0 lands; all of
            # x is prefetched up front, balanced across the queues
            for dc in range(KD):
                nc.sync.dma_start(wv_t[:, dc, :], wvr[:, dc, :])
                nc.gpsimd.dma_start(xb[:, dc, 0:512], xTr[:, dc, 0:512])
            cos_t = cpool.tile([128, N], F16, tag="cos")
            sin_t = cpool.tile([128, N], F16, tag="sin")

            ve = [
                cpool.tile([128, PAIRS, MB, 65], F16, tag=f"ve{j}", name=f"ve{j}")
                for j in range(2)
            ]
            for j in range(2):
                nc.gpsimd.memset(ve[j][:, :, :, 64:65], 1.0)
            # ones rows for the denominator-broadcast matmul (K=1, M=64);
            # rows 0 and 32 are used (matching rec's row layout). The value
            # is 1/64 because rec is prescaled by 64 to keep 64/D in fp16's
            # normal range (1/D can go subnormal); bc = (1/64)*(64/D) = 1/D.
            ones16 = cpool.tile([33, 64], F16, tag="ones16")
            nc.vector.memset(ones16[:], 1.0 / 64.0)

            # ---- per-pair q/k projection blocks ----
            def proj_pair_blocks(p):
                csl = slice(p * 128, (p + 1) * 128)
                wt = {}

                def load_w():
                    for nm, wd in (("q", wq), ("k", wk)):
                        t = iopool.tile(
                            [128, KD, 128], F16, tag=f"w{nm}", bufs=2, name=f"w{nm}_{p}"
                        )
                        nc.sync.dma_start(
                            t[:], wd.rearrange("(c p) i -> p c i", p=128)[:, :, csl]
                        )
                        wt[nm] = t

                qT_t = qkpool.tile([128, N], F16, tag="qT", bufs=2)
                kT_t = qkpool.tile([128, N], F16, tag="kT", bufs=2)

                def block(nb, nm, tgt):
                    def emit():
                        nsl = slice(nb * 512, (nb + 1) * 512)
                        pq = ps.tile([128, 2, 512], F32, tag="s", bufs=3)
                        for dc in range(KD):
                            nc.tensor.matmul(
                                pq[:, 0, :], wt[nm][:, dc, :], xb[:, dc, nsl],
                                start=(dc == 0), stop=(dc == KD - 1),
                            )
                        # rotate_half via 32-partition shifted copies; sign
                        # folded into sin_t (host negates alternate groups)
                        tmp = iopool.tile([128, 512], F16, tag="tmp", bufs=2)
                        for g in range(4):
                            dst = slice(g * 32, (g + 1) * 32)
                            ssrc = slice((g ^ 1) * 32, ((g ^ 1) + 1) * 32)
                            nc.vector.tensor_copy(tmp[dst, :], pq[ssrc, 0, :])
                        nc.vector.tensor_mul(tmp[:], tmp[:], sin_t[:, nsl])
                        nc.vector.tensor_mul(tgt[:, nsl], pq[:, 0, :], cos_t[:, nsl])
                        nc.vector.tensor_add(tgt[:, nsl], tgt[:, nsl], tmp[:])

                    return emit

                blocks = {}
                for nb in range(NB):
                    blocks[("q", nb)] = block(nb, "q", qT_t)
                    blocks[("k", nb)] = block(nb, "k", kT_t)
                return load_w, blocks, qT_t, kT_t

            with tc.tile_pool(name="attn", bufs=1) as at:
                otn = [
                    at.tile([128, 4, 512], F16, tag=f"otn{p}", name=f"otn{p}")
                    for p in range(PAIRS)
                ]
                wo_h = []

                def load_wo():
                    for dh in range(2):
                        woh = qkpool.tile(
                            [128, PAIRS, 512], F16, tag=("qT", "kT")[dh], bufs=2,
                            name=f"wo_h{dh}",
                        )
                        nc.sync.dma_start(
                            woh[:],
                            wo.rearrange("(c p) d -> p c d", p=128)[
                                :, :, dh * 512 : (dh + 1) * 512
                            ],
                        )
                        wo_h.append(woh)

                fillers = []

                def drain(k=1):
                    for _ in range(k):
                        if fillers:
                            fillers.pop(0)()

                pending = []  # deferred normalize tails

                def run_pending():
                    while pending:
                        pending.pop(0)()

                def outproj_block(nbk, dh):
                    def emit():
                        q4, r4 = divmod(nbk, 4)
                        nsl = slice(nbk * 128, (nbk + 1) * 128)
                        po = ps.tile([128, 2, 512], F32, tag="s", bufs=3)
                        for c in range(PAIRS):
                            nc.tensor.matmul(
                                po[:, 0, :],
                                otn[c][:, q4, r4 * 128 : (r4 + 1) * 128],
                                wo_h[dh][:, c, :],
                                start=(c == 0),
                                stop=(c == PAIRS - 1),
                            )
                        ost = iopool.tile([128, 512], F32, tag="ost", bufs=2)
                        nc.vector.tensor_copy(ost[:], po[:, 0, :])
                        nc.sync.dma_start(
                            out[nsl, dh * 512 : (dh + 1) * 512], ost[:]
                        )

                    return emit

                # ---- attention step: exp j0 is issued before scores j1 so
                # the scalar engine gets a head start on the av matmuls ----
                def scores_exp_av(p, qi, mb2, qT_t, kT_t, ot_ab):
                    n0 = qi * 512
                    pts = []
                    for j in range(2):
                        psl = slice(64 * j, 64 * (j + 1))
                        s_t = ps.tile(
                            [128, 2, 512], F32, tag="s", bufs=3, name=f"s{j}"
                        )
                        for hm in range(2):
                            mb = 2 * mb2 + hm
                            msl = slice(mb * 128, (mb + 1) * 128)
                            nc.tensor.matmul(
                                s_t[:, hm, :],
                                kT_t[psl, msl],
                                qT_t[psl, n0 : n0 + 512],
                                start=True,
                                stop=True,
                            )
                        pt = at.tile(
                            [128, 2, 512], F16, tag="pt", bufs=5, name=f"pt{j}"
                        )
                        nc.scalar.activation(pt[:], s_t[:], EXP, scale=SCALE)
                        pts.append(pt)
                    for j in range(2):
                        for hm in range(2):
                            mb = 2 * mb2 + hm
                            nc.tensor.matmul(
                                ot_ab[j][0:65, :],
                                ve[j][:, p, mb, :],
                                pts[j][:, hm, :],
                                start=(mb == 0),
                                stop=(mb == MB - 1),
                            )

                def spill_quarter(p, qi, ot_ab, last=False):
                    """Copy accumulators off PSUM at quarter end; defer the
                    reciprocal/broadcast/scale tail off the critical path."""
                    osb = at.tile([65, 2, 512], F16, tag="ots", bufs=2)
                    nc.vector.tensor_copy(osb[:, 0, :], ot_ab[0][0:65, :])
                    nc.vector.tensor_copy(osb[:, 1, :], ot_ab[1][0:65, :])

                    def tail():
                        rin = at.tile([33, 512], F32, tag="rin", bufs=2)
                        nc.vector.tensor_copy(rin[0:1, :], osb[64:65, 0, :])
                        nc.vector.tensor_copy(rin[32:33, :], osb[64:65, 1, :])
                        rec = at.tile([33, 512], F32R, tag="rec", bufs=2)
                        with nc.allow_low_precision(
                            reason="f32r reciprocal for softmax denom"
                        ):
                            nc.vector.reciprocal(rec[:], rin[:])
                        rec16 = at.tile([33, 512], F16, tag="rec16", bufs=2)
                        nc.vector.tensor_scalar_mul(rec16[:], rec[:], 64.0)
                        bcs = ps.tile(
                            [128, 2, 512], F32, tag="s", bufs=3, name="bcs"
                        )
                        for j in range(2):
                            row = 32 * j
                            nc.tensor.matmul(
                                bcs[0:64, j, :],
                                ones16[row : row + 1, :],
                                rec16[row : row + 1, :],
                                start=True,
                                stop=True,
                            )
                        for j in range(2):
                            nc.vector.tensor_mul(
                                otn[p][64 * j : 64 * (j + 1), qi, :],
                                osb[0:64, j, :],
                                bcs[0:64, j, :],
                            )
                        if p == PAIRS - 1:
                            for r4 in range(4):
                                for dh in range(2):
                                    fillers.append(outproj_block(qi * 4 + r4, dh))

                    def tail_last():
                        # split in n-halves so the final out-projections
                        # pipeline with the reciprocal chain
                        rin = at.tile([33, 512], F32, tag="rin", bufs=2)
                        nc.vector.tensor_copy(rin[0:1, :], osb[64:65, 0, :])
                        nc.vector.tensor_copy(rin[32:33, :], osb[64:65, 1, :])
                        rec = at.tile([33, 512], F32R, tag="rec", bufs=2)
                        rec16 = at.tile([33, 512], F16, tag="rec16", bufs=2)
                        bcs = ps.tile(
                            [128, 2, 512], F32, tag="s", bufs=3, name="bcs"
                        )
                        for h in range(2):
                            hs = slice(h * 256, (h + 1) * 256)
                            with nc.allow_low_precision(
                                reason="f32r reciprocal for softmax denom"
                            ):
                                nc.vector.reciprocal(rec[:, hs], rin[:, hs])
                            nc.vector.tensor_scalar_mul(
                                rec16[:, hs], rec[:, hs], 64.0
                            )
                            for j in range(2):
                                row = 32 * j
                                nc.tensor.matmul(
                                    bcs[0:64, j, hs],
                                    ones16[row : row + 1, :],
                                    rec16[row : row + 1, hs],
                                    start=True,
                                    stop=True,
                                )
                                nc.vector.tensor_mul(
                                    otn[p][64 * j : 64 * (j + 1), qi, hs],
                                    osb[0:64, j, hs],
                                    bcs[0:64, j, hs],
                                )
                            for r4 in (2 * h, 2 * h + 1):
                                for dh in range(2):
                                    outproj_block(qi * 4 + r4, dh)()

                    if last:
                        tail_last()
                    else:
                        pending.append(tail)

                # ---- pair 0: merged v/k/q pass + quarter-0 attention ----
                load_w0, blocks0, qT0, kT0 = proj_pair_blocks(0)
                load_w0()
                # prefetch the rest of x + rope tables behind the weights,
                # spread across four DMA queues (vector/scalar are idle here)
                nc.sync.dma_start(cos_t[:], cosT[:])
                nc.gpsimd.dma_start(sin_t[:], sinT[:])
                nc.scalar.dma_start(xb[:, :, 512:1024], xTr[:, :, 512:1024])
                nc.gpsimd.dma_start(xb[:, :, 1024:1536], xTr[:, :, 1024:1536])
                nc.sync.dma_start(xb[:, :, 1536:2048], xTr[:, :, 1536:2048])
                ot_q0 = [
                    ps.tile([128, 512], F32, tag="ot", bufs=2, name=f"ot{jj}")
                    for jj in range(2)
                ]
                # q blocks are emitted a quarter ahead of their use; the
                # quarter-0 attention lags one nb behind the k/v it needs so
                # scores never wait on just-emitted RoPE work
                q_sched = {0: [0], 1: [], 2: [1], 3: [2]}
                attn_sched = {0: (), 1: (0, 1), 2: (2, 3), 3: (4, 5)}
                for nb in range(NB):
                    for s2 in range(2):
                        pv = ps.tile([128, 2, 512], F32, tag="s", bufs=3)
                        for half in range(2):
                            mb = nb * 4 + s2 * 2 + half
                            m0 = mb * 128
                            for dc in range(KD):
                                nc.tensor.matmul(
                                    pv[:, half, :],
                                    xb[:, dc, m0 : m0 + 128],
                                    wv_t[:, dc, :],
                                    start=(dc == 0),
                                    stop=(dc == KD - 1),
                                )
                        pvr = pv[:].rearrange(
                            "p h (c j d) -> p h c j d", c=4, j=2, d=64
                        )
                        for half in range(2):
                            mb = nb * 4 + s2 * 2 + half
                            for j in range(2):
                                # scalar engine is idle until exp ramps up
                                nc.scalar.copy(
                                    ve[j][:, :, mb, 0:64], pvr[:, half, :, j, :]
                                )
                    blocks0[("k", nb)]()
                    for qb in q_sched[nb]:
                        blocks0[("q", qb)]()
                    for mb2 in attn_sched[nb]:
                        scores_exp_av(0, 0, mb2, qT0, kT0, ot_q0)
                for mb2 in (6, 7):
                    scores_exp_av(0, 0, mb2, qT0, kT0, ot_q0)
                spill_quarter(0, 0, ot_q0)

                pair_qk = {0: (qT0, kT0, blocks0)}

                # ---- remaining quarters / pairs ----
                for p in range(PAIRS):
                    qT_t, kT_t, blocks_p = pair_qk.pop(p)
                    if p == PAIRS - 1:
                        load_wo()
                    if p + 1 < PAIRS:
                        load_wn, blocks_n, qTn, kTn = proj_pair_blocks(p + 1)
                        load_wn()
                        pair_qk[p + 1] = (qTn, kTn, blocks_n)
                        new_fill = [
                            blocks_n[(nm, nb)] for nb in range(NB) for nm in ("k", "q")
                        ]
                        if p == 0:
                            # pair 0's own last q block goes first
                            new_fill.insert(0, blocks_p[("q", 3)])
                        fillers.extend(new_fill)
                    quarters = range(1, 4) if p == 0 else range(4)
                    for qi in quarters:
                        if p == 0:
                            drain_at = (1, 4, 6)
                            dk = 1
                        elif p < PAIRS - 1:
                            drain_at = (2, 5)
                            dk = 1
                        else:
                            drain_at = (1, 2, 3, 5, 6, 7)
                            dk = 2
                        ot_ab = [
                            ps.tile(
                                [128, 512], F32, tag="ot", bufs=2, name=f"ot{jj}"
                            )
                            for jj in range(2)
                        ]
                        for mb2 in range(MB // 2):
                            scores_exp_av(p, qi, mb2, qT_t, kT_t, ot_ab)
                            if mb2 == 4:
                                run_pending()
                            if mb2 in drain_at:
                                drain(dk)
                        spill_quarter(
                            p, qi, ot_ab, last=(p == PAIRS - 1 and qi == 3)
                        )
                        if p == PAIRS - 1 and qi == 3:
                            drain(len(fillers))

    return nc


_CACHED = {}


def _get_kernel():
    if "nc" not in _CACHED:
        _CACHED["nc"] = build_kernel()
    return _CACHED["nc"]


def kernel(x, rotary_emb_x, Wq, Wkv, Wo, bo):
    from concourse.bass_utils import run_bass_kernel_spmd

    x = np.asarray(x, np.float32)
    rope = np.asarray(rotary_emb_x, np.float32)
    Wq = np.asarray(Wq, np.float32)
    Wkv = np.asarray(Wkv, np.float32)
    Wo = np.asarray(Wo, np.float32)
    bo = np.asarray(bo, np.float32)

    cosT = np.ascontiguousarray(np.cos(rope).T)  # [64, N]
    sinT = np.ascontiguousarray(np.sin(rope).T)
    cosT2 = np.concatenate([cosT, cosT], axis=0)
    sinT2 = np.concatenate([sinT, sinT], axis=0)
    # fold rotate_half's sign into sin: the low half of each 64-row head
    # block multiplies -q_hi
    sinT2 = sinT2.copy()
    sinT2[0:32] = -sinT2[0:32]
    sinT2[64:96] = -sinT2[64:96]
    cosT2 = np.ascontiguousarray(cosT2.astype(np.float16))
    sinT2 = np.ascontiguousarray(sinT2.astype(np.float16))

    Wk_full = Wkv[:, : H * DH]
    Wv_full = Wkv[:, H * DH :]

    xTs = [np.ascontiguousarray(x[b].T.astype(np.float16)) for b in range(B)]
    in_maps = []
    for core in range(N_CORES):
        b, hg = divmod(core, 2)
        isl = slice(hg * INNER, (hg + 1) * INNER)
        in_maps.append(
            {
                "xT": xTs[b],
                "wq": np.ascontiguousarray(Wq[:, isl].astype(np.float16)),
                "wk": np.ascontiguousarray(Wk_full[:, isl].astype(np.float16)),
                "wv": np.ascontiguousarray(Wv_full[:, isl].astype(np.float16)),
                "wo": np.ascontiguousarray(Wo[isl, :].astype(np.float16)),
                "cosT": cosT2,
                "sinT": sinT2,
            }
        )

    nc = _get_kernel()
    _CACHED["in_maps"] = in_maps
    res = run_bass_kernel_spmd(nc, in_maps, list(range(N_CORES)))
    outs = [res.results[i]["out"] for i in range(N_CORES)]
    full = np.stack(
        [outs[2 * b] + outs[2 * b + 1] + bo for b in range(B)], axis=0
    )
    return full
